# revision 3
# baseline (speedup 1.0000x reference)
"""Trainium2 Bass kernel for the gated-attention multi-bag SSL head.

Computation (eval mode):
    H   = relu(x @ W1 + b1)                      [N, D]
    a   = (tanh(H@Wt+bt) * sigmoid(H@Ws+bs)) @ Wa + ba
    w   = segment_softmax(a, idxs)               (idxs sorted, 256 bags)
    M   = segment_sum(w * H)                     [B, D]
    proj= l2norm(M @ Wp + bp)                    [B, F]

Device strategy (8 NeuronCores, data-parallel over the instance dim N):
  * x is transposed on the host so the contraction dim L lands on SBUF
    partitions; each core gets a contiguous [L, N/8] shard.
  * Softmax skips the segment-max: |a| <= F*max|Wa| ~ 6, so exp(a) is
    safe in fp32 and exp(a)/sum(exp(a)) == softmax(a).  This makes the
    whole kernel single-pass: each core accumulates U[b] = sum e_i*H_i
    and den[b] = sum e_i in one PSUM bank via one-hot matmuls.
  * sigmoid(z) = 0.5*(1+tanh(z/2)); the 0.5 folds into Wa on the host.
    This keeps every activation (relu/tanh/exp) in ONE ACT table set.
  * Host combines per-core U/den (adjacent shards share at most one
    bag) and runs the tiny [256,128] projector epilogue.
"""

import numpy as np

N_CORES = 8
L, D, F, NBAGS = 1024, 128, 32, 256
N_TOTAL = 262144
SLICE = 512
CH_COLS = 2048
UW = D + 1  # U columns: 128 H-dims + 1 density column
KCH = L // 128  # 8 contraction chunks

_CACHE = {}


def _build(n_rows, ch_cols):
    from contextlib import ExitStack

    import concourse.bacc as bacc
    import concourse.tile as tile
    from concourse import mybir

    F32 = mybir.dt.float32
    F32R = mybir.dt.float32r
    AF = mybir.ActivationFunctionType
    OP = mybir.AluOpType

    assert n_rows % ch_cols == 0 and ch_cols % SLICE == 0
    n_ch = n_rows // ch_cols
    sl_per_ch = ch_cols // SLICE
    grp_per_sl = SLICE // 128
    n_grp = n_rows // 128
    n_u_mm = n_ch * sl_per_ch * grp_per_sl

    nc = bacc.Bacc(
        "TRN2", target_bir_lowering=False, debug=False, num_devices=N_CORES
    )
    xt = nc.dram_tensor("xt", [L, n_rows], F32R, kind="ExternalInput").ap()
    idx = nc.dram_tensor("idx", [128, n_grp], F32, kind="ExternalInput").ap()
    iota = nc.dram_tensor("iota", [128, 128], F32, kind="ExternalInput").ap()
    ident = nc.dram_tensor("ident", [128, 128], F32R, kind="ExternalInput").ap()
    w1 = nc.dram_tensor("w1", [L, D], F32R, kind="ExternalInput").ap()
    wt = nc.dram_tensor("wt", [D, F], F32R, kind="ExternalInput").ap()
    ws = nc.dram_tensor("ws", [D, F], F32R, kind="ExternalInput").ap()
    wa2 = nc.dram_tensor("wa2", [F, 1], F32, kind="ExternalInput").ap()
    b1c = nc.dram_tensor("b1c", [128, 1], F32, kind="ExternalInput").ap()
    btc = nc.dram_tensor("btc", [F, 1], F32, kind="ExternalInput").ap()
    bsc2 = nc.dram_tensor("bsc2", [F, 1], F32, kind="ExternalInput").ap()
    bac = nc.dram_tensor("bac", [128, 1], F32, kind="ExternalInput").ap()
    u_out = nc.dram_tensor("u", [128, UW], F32, kind="ExternalOutput").ap()

    xt_r = xt.rearrange("(a p) n -> p a n", p=128)  # [128, KCH, n_rows]

    with tile.TileContext(nc) as tc, ExitStack() as ctx:
        const = ctx.enter_context(tc.tile_pool(name="const", bufs=1))
        xpool = ctx.enter_context(tc.tile_pool(name="xin", bufs=2))
        htp = ctx.enter_context(tc.tile_pool(name="htp", bufs=3))
        scp = ctx.enter_context(tc.tile_pool(name="scp", bufs=2))
        wop = ctx.enter_context(tc.tile_pool(name="wop", bufs=3))
        hnp = ctx.enter_context(tc.tile_pool(name="hnp", bufs=3))
        epool = ctx.enter_context(tc.tile_pool(name="ep", bufs=2))
        outp = ctx.enter_context(tc.tile_pool(name="outp", bufs=1))
        ph = ctx.enter_context(tc.tile_pool(name="ph", bufs=2, space="PSUM"))
        pts = ctx.enter_context(tc.tile_pool(name="pts", bufs=2, space="PSUM"))
        pa = ctx.enter_context(tc.tile_pool(name="pa", bufs=1, space="PSUM"))
        ptrp = ctx.enter_context(tc.tile_pool(name="ptrp", bufs=1, space="PSUM"))
        pu = ctx.enter_context(tc.tile_pool(name="pu", bufs=1, space="PSUM"))

        # ---- constants ----
        w1sb = []
        for k in range(KCH):
            t = const.tile([128, D], F32R, tag=f"w1_{k}")
            nc.sync.dma_start(t[:], w1[k * 128 : (k + 1) * 128, :])
            w1sb.append(t)
        wtsb = const.tile([D, F], F32R, tag="wt")
        nc.sync.dma_start(wtsb[:], wt[:])
        wssb = const.tile([D, F], F32R, tag="ws")
        nc.sync.dma_start(wssb[:], ws[:])
        wasb = const.tile([F, 1], F32, tag="wa")
        nc.sync.dma_start(wasb[:], wa2[:])
        idxsb = const.tile([128, n_grp], F32, tag="idx")
        nc.sync.dma_start(idxsb[:], idx[:])
        iotasb = const.tile([128, 128], F32, tag="iota")
        nc.sync.dma_start(iotasb[:], iota[:])
        identsb = const.tile([128, 128], F32R, tag="ident")
        nc.sync.dma_start(identsb[:], ident[:])
        b1sb = const.tile([128, 1], F32, tag="b1")
        nc.sync.dma_start(b1sb[:], b1c[:])
        btsb = const.tile([F, 1], F32, tag="bt")
        nc.sync.dma_start(btsb[:], btc[:])
        bssb = const.tile([F, 1], F32, tag="bs")
        nc.sync.dma_start(bssb[:], bsc2[:])
        basb = const.tile([128, 1], F32, tag="ba")
        nc.sync.dma_start(basb[:], bac[:])

        psum_u = pu.tile([128, UW], F32)
        u_mm = 0

        for ch in range(n_ch):
            xk = xpool.tile([128, KCH, ch_cols], F32R, tag="xk")
            nc.sync.dma_start(
                xk[:], xt_r[:, :, ch * ch_cols : (ch + 1) * ch_cols]
            )
            for s in range(sl_per_ch):
                c0 = s * SLICE
                # H^T[d, n] accumulation over the 8 L-chunks
                psum_h = ph.tile([128, SLICE], F32, tag="psh")
                for k in range(KCH):
                    nc.tensor.matmul(
                        psum_h[:],
                        w1sb[k][:],
                        xk[:, k, c0 : c0 + SLICE],
                        start=(k == 0),
                        stop=(k == KCH - 1),
                    )
                ht = htp.tile([128, SLICE], F32R, tag="ht")
                nc.scalar.activation(ht[:], psum_h[:], AF.Relu, bias=b1sb[:])

                # gated attention scores (transposed layout [F, n])
                pt = pts.tile([F, SLICE], F32, tag="pts")
                ps = pts.tile([F, SLICE], F32, tag="pts")
                nc.tensor.matmul(
                    pt[:], wtsb[:], ht[:],
                    start=True, stop=True,
                )
                nc.tensor.matmul(
                    ps[:], wssb[:], ht[:],
                    start=True, stop=True,
                )
                at = scp.tile([F, SLICE], F32, tag="at")
                nc.scalar.activation(at[:], pt[:], AF.Tanh, bias=btsb[:])
                ts = scp.tile([F, SLICE], F32, tag="ts")
                nc.scalar.activation(
                    ts[:], ps[:], AF.Tanh, bias=bssb[:], scale=0.5
                )
                g = scp.tile([F, SLICE], F32, tag="g")
                nc.vector.scalar_tensor_tensor(
                    g[:], ts[:], 1.0, at[:], op0=OP.add, op1=OP.mult
                )

                # a[n] as 128-row columns, then e = exp(a + ba)
                pat = pa.tile([128, grp_per_sl], F32, tag="pat")
                for j in range(grp_per_sl):
                    nc.tensor.matmul(
                        pat[:, j : j + 1],
                        g[:, j * 128 : (j + 1) * 128],
                        wasb[:],
                        start=True,
                        stop=True,
                    )
                e = epool.tile([128, grp_per_sl], F32, tag="e")
                nc.scalar.activation(e[:], pat[:], AF.Exp, bias=basb[:])

                # transpose H back to natural layout, build weighted
                # one-hot lhsT, accumulate U/den
                ptr_t = ptrp.tile([128, SLICE], F32R, tag="ptr")
                for j in range(grp_per_sl):
                    gcol = ch * (ch_cols // 128) + s * grp_per_sl + j
                    nc.tensor.transpose(
                        ptr_t[:, j * 128 : (j + 1) * 128],
                        ht[:, j * 128 : (j + 1) * 128],
                        identsb[:],
                    )
                    hn = hnp.tile([128, UW], F32, tag="hn")
                    nc.vector.tensor_copy(
                        hn[:, 0:D], ptr_t[:, j * 128 : (j + 1) * 128]
                    )
                    nc.vector.memset(hn[:, D : D + 1], 1.0)
                    wo = wop.tile([128, 128], F32, tag="wo")
                    nc.vector.tensor_scalar(
                        wo[:],
                        iotasb[:],
                        idxsb[:, gcol : gcol + 1],
                        e[:, j : j + 1],
                        op0=OP.is_equal,
                        op1=OP.mult,
                    )
                    nc.tensor.matmul(
                        psum_u[:],
                        wo[:],
                        hn[:],
                        start=(u_mm == 0),
                        stop=(u_mm == n_u_mm - 1),
                    )
                    u_mm += 1

        u_sb = outp.tile([128, UW], F32, tag="usb")
        nc.vector.tensor_copy(u_sb[:], psum_u[:])
        nc.sync.dma_start(u_out[:], u_sb[:])

    nc.compile()
    return nc


def _get_nc(n_rows, ch_cols):
    key = (n_rows, ch_cols)
    if key not in _CACHE:
        _CACHE[key] = _build(n_rows, ch_cols)
    return _CACHE[key]


def _host_prep(x, idxs, W1, b1, Wt, bt, Ws, bs, Wa, ba):
    n_rows = x.shape[0] // N_CORES
    n_grp = n_rows // 128
    xT = np.ascontiguousarray(x.T)  # [L, N]
    shared = {
        "iota": np.broadcast_to(
            np.arange(128, dtype=np.float32)[None, :], (128, 128)
        ).copy(),
        "ident": np.eye(128, dtype=np.float32),
        "w1": np.ascontiguousarray(W1, np.float32),
        "wt": np.ascontiguousarray(Wt, np.float32),
        "ws": np.ascontiguousarray(Ws, np.float32),
        "wa2": np.ascontiguousarray(Wa * 0.5, np.float32),
        "b1c": np.ascontiguousarray(b1.reshape(128, 1), np.float32),
        "btc": np.ascontiguousarray(bt.reshape(F, 1), np.float32),
        "bsc2": np.ascontiguousarray(0.5 * bs.reshape(F, 1), np.float32),
        "bac": np.full((128, 1), float(np.asarray(ba).reshape(-1)[0]), np.float32),
    }
    in_maps = []
    bases = []
    for c in range(N_CORES):
        lo, hi = c * n_rows, (c + 1) * n_rows
        base = int(idxs[lo])
        span = int(idxs[hi - 1]) - base + 1
        if span > 128:
            return None, None  # triggers numpy fallback
        bases.append(base)
        idl = (idxs[lo:hi] - base).astype(np.float32)
        m = dict(shared)
        m["xt"] = np.ascontiguousarray(xT[:, lo:hi])
        m["idx"] = np.ascontiguousarray(idl.reshape(n_grp, 128).T)
        in_maps.append(m)
    return in_maps, bases


def _combine(results, bases, Wp, bp):
    U_full = np.zeros((NBAGS + 128, D), np.float64)
    den_full = np.zeros(NBAGS + 128, np.float64)
    for c in range(N_CORES):
        u = results[c]["u"]
        U_full[bases[c] : bases[c] + 128] += u[:, :D]
        den_full[bases[c] : bases[c] + 128] += u[:, D]
    U_full = U_full[:NBAGS]
    den_full = den_full[:NBAGS]
    den_safe = np.where(den_full == 0, 1.0, den_full)
    M = (U_full / den_safe[:, None]).astype(np.float32)
    proj = (M @ np.asarray(Wp, np.float32) + np.asarray(bp, np.float32)).astype(
        np.float32
    )
    nrm = np.maximum(np.linalg.norm(proj, axis=1, keepdims=True), 1e-12)
    proj = (proj / nrm).astype(np.float32)
    return M, proj


def _numpy_fallback(x, idxs, W1, b1, Wt, bt, Ws, bs, Wa, ba, Wp, bp):
    H = np.maximum(x @ W1 + b1, 0.0).astype(np.float32)
    At = np.tanh(H @ Wt + bt)
    As = 1.0 / (1.0 + np.exp(-(H @ Ws + bs)))
    a = ((At * As) @ Wa)[:, 0] + np.asarray(ba).reshape(-1)[0]
    a = a - a.max()
    e = np.exp(a)
    den = np.zeros(NBAGS)
    np.add.at(den, idxs, e)
    U = np.zeros((NBAGS, D))
    np.add.at(U, idxs, e[:, None] * H)
    den = np.where(den == 0, 1.0, den)
    M = (U / den[:, None]).astype(np.float32)
    proj = (M @ Wp + bp).astype(np.float32)
    nrm = np.maximum(np.linalg.norm(proj, axis=1, keepdims=True), 1e-12)
    return M, (proj / nrm).astype(np.float32)


def kernel(x, idxs, W1, b1, Wt, bt, Ws, bs, Wa, ba, Wp, bp):
    from concourse.bass_utils import run_bass_kernel_spmd

    x = np.ascontiguousarray(np.asarray(x), np.float32)
    idxs = np.asarray(idxs).astype(np.int64)
    args = [np.asarray(v, np.float32) for v in (W1, b1, Wt, bt, Ws, bs, Wa, ba)]
    W1, b1, Wt, bt, Ws, bs, Wa, ba = args
    Wp = np.asarray(Wp, np.float32)
    bp = np.asarray(bp, np.float32)

    in_maps, bases = _host_prep(x, idxs, W1, b1, Wt, bt, Ws, bs, Wa, ba)
    if in_maps is None:
        return _numpy_fallback(
            x, idxs, W1, b1, Wt, bt, Ws, bs, Wa, ba, Wp, bp
        )
    n_rows = x.shape[0] // N_CORES
    nc = _get_nc(n_rows, CH_COLS)
    res = run_bass_kernel_spmd(nc, in_maps, list(range(N_CORES)), trace=False)
    return _combine(res.results, bases, Wp, bp)


# revision 7
# speedup vs baseline: 1.0887x; 1.0887x over previous
"""Trainium2 Bass kernel for the gated-attention multi-bag SSL head.

Computation (eval mode):
    H   = relu(x @ W1 + b1)                      [N, D]
    a   = (tanh(H@Wt+bt) * sigmoid(H@Ws+bs)) @ Wa + ba
    w   = segment_softmax(a, idxs)               (idxs sorted, 256 bags)
    M   = segment_sum(w * H)                     [B, D]
    proj= l2norm(M @ Wp + bp)                    [B, F]

Device strategy (8 NeuronCores, data-parallel over the instance dim N):
  * x is transposed on the host so the contraction dim L lands on SBUF
    partitions; each core gets a contiguous [L, N/8] shard.
  * Softmax skips the segment-max: |a| <= F*max|Wa| ~ 6, so exp(a) is
    safe in fp32 and exp(a)/sum(exp(a)) == softmax(a).  This makes the
    whole kernel single-pass: each core accumulates U[b] = sum e_i*H_i
    and den[b] = sum e_i in one PSUM bank via one-hot matmuls.
  * sigmoid(z) = 0.5*(1+tanh(z/2)); the 0.5 folds into Wa on the host.
    This keeps every activation (relu/tanh/exp) in ONE ACT table set.
  * Host combines per-core U/den (adjacent shards share at most one
    bag) and runs the tiny [256,128] projector epilogue.
"""

import numpy as np

N_CORES = 8
L, D, F, NBAGS = 1024, 128, 32, 256
N_TOTAL = 262144
SLICE = 512
CH_COLS = 2048
UW = D + 1  # U columns: 128 H-dims + 1 density column
UPAD = 256  # padded U-matmul width so float32r streams at 1 cyc/row
KCH = L // 128  # 8 contraction chunks

_CACHE = {}


def _build(n_rows, ch_cols):
    from contextlib import ExitStack

    import concourse.bacc as bacc
    import concourse.tile as tile
    from concourse import mybir

    F32 = mybir.dt.float32
    F32R = mybir.dt.float32r
    AF = mybir.ActivationFunctionType
    OP = mybir.AluOpType

    assert n_rows % ch_cols == 0 and ch_cols % SLICE == 0
    n_ch = n_rows // ch_cols
    sl_per_ch = ch_cols // SLICE
    grp_per_sl = SLICE // 128
    n_grp = n_rows // 128
    n_u_mm = n_ch * sl_per_ch * grp_per_sl

    nc = bacc.Bacc(
        "TRN2", target_bir_lowering=False, debug=False, num_devices=N_CORES
    )
    xt = nc.dram_tensor("xt", [L, n_rows], F32R, kind="ExternalInput").ap()
    idx = nc.dram_tensor("idx", [128, n_grp], F32, kind="ExternalInput").ap()
    iota = nc.dram_tensor("iota", [128, 128], F32, kind="ExternalInput").ap()
    ident = nc.dram_tensor("ident", [128, 128], F32R, kind="ExternalInput").ap()
    w1 = nc.dram_tensor("w1", [L, D], F32R, kind="ExternalInput").ap()
    wt = nc.dram_tensor("wt", [D, F], F32R, kind="ExternalInput").ap()
    ws = nc.dram_tensor("ws", [D, F], F32R, kind="ExternalInput").ap()
    wa2 = nc.dram_tensor("wa2", [F, 2], F32R, kind="ExternalInput").ap()
    b1c = nc.dram_tensor("b1c", [128, 1], F32, kind="ExternalInput").ap()
    btc = nc.dram_tensor("btc", [F, 1], F32, kind="ExternalInput").ap()
    bsc2 = nc.dram_tensor("bsc2", [F, 1], F32, kind="ExternalInput").ap()
    bac = nc.dram_tensor("bac", [128, 1], F32, kind="ExternalInput").ap()
    u_out = nc.dram_tensor("u", [128, UW], F32, kind="ExternalOutput").ap()

    xt_r = xt.rearrange("(a p) n -> p a n", p=128)  # [128, KCH, n_rows]

    with tile.TileContext(nc) as tc, ExitStack() as ctx:
        const = ctx.enter_context(tc.tile_pool(name="const", bufs=1))
        xpool = ctx.enter_context(tc.tile_pool(name="xin", bufs=2))
        htp = ctx.enter_context(tc.tile_pool(name="htp", bufs=3))
        scp = ctx.enter_context(tc.tile_pool(name="scp", bufs=2))
        wop = ctx.enter_context(tc.tile_pool(name="wop", bufs=3))
        hnp = ctx.enter_context(tc.tile_pool(name="hnp", bufs=3))
        epool = ctx.enter_context(tc.tile_pool(name="ep", bufs=2))
        outp = ctx.enter_context(tc.tile_pool(name="outp", bufs=1))
        ph = ctx.enter_context(tc.tile_pool(name="ph", bufs=2, space="PSUM"))
        pts = ctx.enter_context(tc.tile_pool(name="pts", bufs=2, space="PSUM"))
        pa = ctx.enter_context(tc.tile_pool(name="pa", bufs=1, space="PSUM"))
        ptrp = ctx.enter_context(tc.tile_pool(name="ptrp", bufs=1, space="PSUM"))
        pu = ctx.enter_context(tc.tile_pool(name="pu", bufs=1, space="PSUM"))

        # ---- constants ----
        w1sb = []
        for k in range(KCH):
            t = const.tile([128, D], F32R, tag=f"w1_{k}")
            nc.sync.dma_start(t[:], w1[k * 128 : (k + 1) * 128, :])
            w1sb.append(t)
        wtsb = const.tile([D, F], F32R, tag="wt")
        nc.sync.dma_start(wtsb[:], wt[:])
        wssb = const.tile([D, F], F32R, tag="ws")
        nc.sync.dma_start(wssb[:], ws[:])
        wasb = const.tile([F, 2], F32R, tag="wa")
        nc.sync.dma_start(wasb[:], wa2[:])
        idxsb = const.tile([128, n_grp], F32, tag="idx")
        nc.sync.dma_start(idxsb[:], idx[:])
        iotasb = const.tile([128, 128], F32, tag="iota")
        nc.sync.dma_start(iotasb[:], iota[:])
        identsb = const.tile([128, 128], F32R, tag="ident")
        nc.sync.dma_start(identsb[:], ident[:])
        b1sb = const.tile([128, 1], F32, tag="b1")
        nc.sync.dma_start(b1sb[:], b1c[:])
        btsb = const.tile([F, 1], F32, tag="bt")
        nc.sync.dma_start(btsb[:], btc[:])
        bssb = const.tile([F, 1], F32, tag="bs")
        nc.sync.dma_start(bssb[:], bsc2[:])
        basb = const.tile([128, 1], F32, tag="ba")
        nc.sync.dma_start(basb[:], bac[:])

        psum_u = pu.tile([128, UPAD], F32)
        u_mm = 0

        for ch in range(n_ch):
            xk = xpool.tile([128, KCH, ch_cols], F32R, tag="xk")
            nc.sync.dma_start(
                xk[:], xt_r[:, :, ch * ch_cols : (ch + 1) * ch_cols]
            )
            for s in range(sl_per_ch):
                c0 = s * SLICE
                # H^T[d, n] accumulation over the 8 L-chunks
                psum_h = ph.tile([128, SLICE], F32, tag="psh")
                for k in range(KCH):
                    nc.tensor.matmul(
                        psum_h[:],
                        w1sb[k][:],
                        xk[:, k, c0 : c0 + SLICE],
                        start=(k == 0),
                        stop=(k == KCH - 1),
                    )
                ht = htp.tile([128, SLICE], F32R, tag="ht")
                nc.scalar.activation(ht[:], psum_h[:], AF.Relu, bias=b1sb[:])

                # gated attention scores (transposed layout [F, n])
                pt = pts.tile([F, SLICE], F32, tag="pts")
                ps = pts.tile([F, SLICE], F32, tag="pts")
                nc.tensor.matmul(
                    pt[:], wtsb[:], ht[:],
                    start=True, stop=True,
                )
                nc.tensor.matmul(
                    ps[:], wssb[:], ht[:],
                    start=True, stop=True,
                )
                at = scp.tile([F, SLICE], F32, tag="at")
                nc.scalar.activation(at[:], pt[:], AF.Tanh, bias=btsb[:])
                ts = scp.tile([F, SLICE], F32, tag="ts")
                nc.scalar.activation(
                    ts[:], ps[:], AF.Tanh, bias=bssb[:], scale=0.5
                )
                g = scp.tile([F, SLICE], F32R, tag="g")
                nc.vector.scalar_tensor_tensor(
                    g[:], ts[:], 1.0, at[:], op0=OP.add, op1=OP.mult
                )

                # a[n] as 128-row columns, then e = exp(a + ba)
                pat = pa.tile([128, 2 * grp_per_sl], F32, tag="pat")
                for j in range(grp_per_sl):
                    nc.tensor.matmul(
                        pat[:, 2 * j : 2 * j + 2],
                        g[:, j * 128 : (j + 1) * 128],
                        wasb[:],
                        start=True,
                        stop=True,
                    )
                e = epool.tile([128, 2 * grp_per_sl], F32, tag="e")
                nc.scalar.activation(e[:], pat[:], AF.Exp, bias=basb[:])

                # transpose H back to natural layout, build weighted
                # one-hot lhsT, accumulate U/den
                ptr_t = ptrp.tile([128, SLICE], F32R, tag="ptr")
                for j in range(grp_per_sl):
                    gcol = ch * (ch_cols // 128) + s * grp_per_sl + j
                    nc.tensor.transpose(
                        ptr_t[:, j * 128 : (j + 1) * 128],
                        ht[:, j * 128 : (j + 1) * 128],
                        identsb[:],
                    )
                    hn = hnp.tile([128, UPAD], F32R, tag="hn")
                    nc.vector.tensor_copy(
                        hn[:, 0:D], ptr_t[:, j * 128 : (j + 1) * 128]
                    )
                    nc.vector.memset(hn[:, D:UPAD].bitcast(F32), 1.0)
                    wo = wop.tile([128, 128], F32R, tag="wo")
                    nc.vector.tensor_scalar(
                        wo[:],
                        iotasb[:],
                        idxsb[:, gcol : gcol + 1],
                        e[:, 2 * j : 2 * j + 1],
                        op0=OP.is_equal,
                        op1=OP.mult,
                    )
                    nc.tensor.matmul(
                        psum_u[:],
                        wo[:],
                        hn[:],
                        start=(u_mm == 0),
                        stop=(u_mm == n_u_mm - 1),
                    )
                    u_mm += 1

        u_sb = outp.tile([128, UW], F32, tag="usb")
        nc.vector.tensor_copy(u_sb[:], psum_u[:, 0:UW])
        nc.sync.dma_start(u_out[:], u_sb[:])

    nc.compile()
    return nc


def _get_nc(n_rows, ch_cols):
    key = (n_rows, ch_cols)
    if key not in _CACHE:
        _CACHE[key] = _build(n_rows, ch_cols)
    return _CACHE[key]


def _host_prep(x, idxs, W1, b1, Wt, bt, Ws, bs, Wa, ba):
    n_rows = x.shape[0] // N_CORES
    n_grp = n_rows // 128
    xT = np.ascontiguousarray(x.T)  # [L, N]
    shared = {
        "iota": np.broadcast_to(
            np.arange(128, dtype=np.float32)[None, :], (128, 128)
        ).copy(),
        "ident": np.eye(128, dtype=np.float32),
        "w1": np.ascontiguousarray(W1, np.float32),
        "wt": np.ascontiguousarray(Wt, np.float32),
        "ws": np.ascontiguousarray(Ws, np.float32),
        "wa2": np.ascontiguousarray(
            np.concatenate(
                [Wa.reshape(F, 1) * 0.5, np.zeros((F, 1), np.float32)], axis=1
            ),
            np.float32,
        ),
        "b1c": np.ascontiguousarray(b1.reshape(128, 1), np.float32),
        "btc": np.ascontiguousarray(bt.reshape(F, 1), np.float32),
        "bsc2": np.ascontiguousarray(0.5 * bs.reshape(F, 1), np.float32),
        "bac": np.full((128, 1), float(np.asarray(ba).reshape(-1)[0]), np.float32),
    }
    in_maps = []
    bases = []
    for c in range(N_CORES):
        lo, hi = c * n_rows, (c + 1) * n_rows
        base = int(idxs[lo])
        span = int(idxs[hi - 1]) - base + 1
        if span > 128:
            return None, None  # triggers numpy fallback
        bases.append(base)
        idl = (idxs[lo:hi] - base).astype(np.float32)
        m = dict(shared)
        m["xt"] = np.ascontiguousarray(xT[:, lo:hi])
        m["idx"] = np.ascontiguousarray(idl.reshape(n_grp, 128).T)
        in_maps.append(m)
    return in_maps, bases


def _combine(results, bases, Wp, bp):
    U_full = np.zeros((NBAGS + 128, D), np.float64)
    den_full = np.zeros(NBAGS + 128, np.float64)
    for c in range(N_CORES):
        u = results[c]["u"]
        U_full[bases[c] : bases[c] + 128] += u[:, :D]
        den_full[bases[c] : bases[c] + 128] += u[:, D]
    U_full = U_full[:NBAGS]
    den_full = den_full[:NBAGS]
    den_safe = np.where(den_full == 0, 1.0, den_full)
    M = (U_full / den_safe[:, None]).astype(np.float32)
    proj = (M @ np.asarray(Wp, np.float32) + np.asarray(bp, np.float32)).astype(
        np.float32
    )
    nrm = np.maximum(np.linalg.norm(proj, axis=1, keepdims=True), 1e-12)
    proj = (proj / nrm).astype(np.float32)
    return M, proj


def _numpy_fallback(x, idxs, W1, b1, Wt, bt, Ws, bs, Wa, ba, Wp, bp):
    H = np.maximum(x @ W1 + b1, 0.0).astype(np.float32)
    At = np.tanh(H @ Wt + bt)
    As = 1.0 / (1.0 + np.exp(-(H @ Ws + bs)))
    a = ((At * As) @ Wa)[:, 0] + np.asarray(ba).reshape(-1)[0]
    a = a - a.max()
    e = np.exp(a)
    den = np.zeros(NBAGS)
    np.add.at(den, idxs, e)
    U = np.zeros((NBAGS, D))
    np.add.at(U, idxs, e[:, None] * H)
    den = np.where(den == 0, 1.0, den)
    M = (U / den[:, None]).astype(np.float32)
    proj = (M @ Wp + bp).astype(np.float32)
    nrm = np.maximum(np.linalg.norm(proj, axis=1, keepdims=True), 1e-12)
    return M, (proj / nrm).astype(np.float32)


def kernel(x, idxs, W1, b1, Wt, bt, Ws, bs, Wa, ba, Wp, bp):
    from concourse.bass_utils import run_bass_kernel_spmd

    x = np.ascontiguousarray(np.asarray(x), np.float32)
    idxs = np.asarray(idxs).astype(np.int64)
    args = [np.asarray(v, np.float32) for v in (W1, b1, Wt, bt, Ws, bs, Wa, ba)]
    W1, b1, Wt, bt, Ws, bs, Wa, ba = args
    Wp = np.asarray(Wp, np.float32)
    bp = np.asarray(bp, np.float32)

    in_maps, bases = _host_prep(x, idxs, W1, b1, Wt, bt, Ws, bs, Wa, ba)
    if in_maps is None:
        return _numpy_fallback(
            x, idxs, W1, b1, Wt, bt, Ws, bs, Wa, ba, Wp, bp
        )
    n_rows = x.shape[0] // N_CORES
    nc = _get_nc(n_rows, CH_COLS)
    res = run_bass_kernel_spmd(nc, in_maps, list(range(N_CORES)), trace=False)
    return _combine(res.results, bases, Wp, bp)
